# revision 11
# baseline (speedup 1.0000x reference)
"""AFNO1D Trainium2 kernel: FFT->block-MLP->softshrink->IFFT->residual.

Strategy: the FFT along C is linear, so it is fused into the layer-1
weights on the host (W1_eff = DFT_block @ w1); the IFFT's real output is
built from half-spectrum matmuls A = o2r@cos, B = o2i@sin with
out[c] = A-B+x and out[1024-c] = A+B+x[1024-c] (reversal done by host
indexing). Everything on-chip is matmul + elementwise, computed
channel-major so the contraction dim sits on SBUF partitions.

Data-parallel over B=8: core b handles x[b]; params replicated; no
collectives. Host transposes shards in/out.

Compute dtype: fp8(e4m3) operands in DoubleRow mode for the two dense
layers, bf16 for the small block-diagonal layer, fp32 PSUM + fp32
residual path. All quantization scales fold into host-prepared weights
and biases; the graph itself has only two fixed descale constants.
"""

from contextlib import ExitStack

import numpy as np
import ml_dtypes

import concourse.bass as bass
import concourse.mybir as mybir
import concourse.tile as tile
from concourse import bacc
from concourse.bass_utils import run_bass_kernel_spmd

HIDDEN = 1024
NB = 8          # channel blocks
BS = 128        # block size
LAM = 0.01
N_CORES = 8
NROWS = 4096    # rows (sequence positions) per core
R = 512         # rows per chunk
NCHUNK = NROWS // R

FP8 = True      # fp8 DoubleRow for layers 1/3 (bf16 fallback if False)
SX = 8.0        # x -> fp8 scale (|x| up to 30 before clipping)
SO = 32.0       # o2 -> fp8 scale (|o2| up to 7.5)
SG = 2048.0     # IFFT cos/sin -> fp8 scale (entries <= 1/32 -> <= 64)
ALU = mybir.AluOpType

F32 = mybir.dt.float32
BF16 = mybir.dt.bfloat16
E4 = mybir.dt.float8e4
DR = mybir.MatmulPerfMode.DoubleRow
RELU = mybir.ActivationFunctionType.Relu

_GRAPH_CACHE = {}


def _build_graph(rep=1, fp8=FP8):
    key = ("nc", rep, fp8)
    if key in _GRAPH_CACHE:
        return _GRAPH_CACHE[key]

    WDT = E4 if fp8 else BF16

    nc = bacc.Bacc("TRN2", target_bir_lowering=False, debug=False,
                   num_devices=N_CORES)

    xt = nc.dram_tensor("xt", [NB, BS, NROWS], F32, kind="ExternalInput").ap()
    w1r = nc.dram_tensor("w1r", [NB, BS, HIDDEN], WDT, kind="ExternalInput").ap()
    w1i = nc.dram_tensor("w1i", [NB, BS, HIDDEN], WDT, kind="ExternalInput").ap()
    gc = nc.dram_tensor("gc", [NB, BS, HIDDEN], WDT, kind="ExternalInput").ap()
    gs = nc.dram_tensor("gs", [NB, BS, HIDDEN], WDT, kind="ExternalInput").ap()
    w20 = nc.dram_tensor("w20", [NB, BS, BS], BF16, kind="ExternalInput").ap()
    w21 = nc.dram_tensor("w21", [NB, BS, BS], BF16, kind="ExternalInput").ap()
    w21n = nc.dram_tensor("w21n", [NB, BS, BS], BF16, kind="ExternalInput").ap()
    b1r = nc.dram_tensor("b1r", [BS, NB], F32, kind="ExternalInput").ap()
    b1i = nc.dram_tensor("b1i", [BS, NB], F32, kind="ExternalInput").ap()
    b2r = nc.dram_tensor("b2r", [BS, NB], F32, kind="ExternalInput").ap()
    b2i = nc.dram_tensor("b2i", [BS, NB], F32, kind="ExternalInput").ap()
    out = nc.dram_tensor("out", [NB, BS, NROWS], F32, kind="ExternalOutput").ap()

    # graph descale constants (everything else is folded host-side)
    K3 = 1.0 / (SO * SG) if fp8 else 1.0
    K5 = 1.0 / (SO * 32.0) if fp8 else 1.0
    LAMS = (SO if fp8 else 1.0) * LAM

    with tile.TileContext(nc) as tc, ExitStack() as ctx:
        wpool = ctx.enter_context(tc.tile_pool(name="weights", bufs=1))
        w1r_sb = wpool.tile([BS, NB, HIDDEN], WDT, tag="w1r", name="w1r_sb")
        w1i_sb = wpool.tile([BS, NB, HIDDEN], WDT, tag="w1i", name="w1i_sb")
        gc_sb = wpool.tile([BS, NB, HIDDEN], WDT, tag="gc", name="gc_sb")
        gs_sb = wpool.tile([BS, NB, HIDDEN], WDT, tag="gs", name="gs_sb")
        for ci in range(NB):
            nc.sync.dma_start(out=w1r_sb[:, ci, :], in_=w1r[ci])
            nc.sync.dma_start(out=w1i_sb[:, ci, :], in_=w1i[ci])
            nc.sync.dma_start(out=gc_sb[:, ci, :], in_=gc[ci])
            nc.sync.dma_start(out=gs_sb[:, ci, :], in_=gs[ci])
        w20_sb = wpool.tile([BS, NB, BS], BF16, tag="w20", name="w20_sb")
        w21_sb = wpool.tile([BS, NB, BS], BF16, tag="w21", name="w21_sb")
        w21n_sb = wpool.tile([BS, NB, BS], BF16, tag="w21n", name="w21n_sb")
        for kb in range(NB):
            nc.sync.dma_start(out=w20_sb[:, kb, :], in_=w20[kb])
            nc.sync.dma_start(out=w21_sb[:, kb, :], in_=w21[kb])
            nc.sync.dma_start(out=w21n_sb[:, kb, :], in_=w21n[kb])
        bias_tiles = {}
        for nm, ap in (("b1r", b1r), ("b1i", b1i), ("b2r", b2r), ("b2i", b2i)):
            t = wpool.tile([BS, NB], F32, tag=nm, name=f"{nm}_sb")
            nc.sync.dma_start(out=t[:], in_=ap[:])
            bias_tiles[nm] = t

        xpool = ctx.enter_context(tc.tile_pool(name="xin", bufs=2))
        bfpool = ctx.enter_context(tc.tile_pool(name="bf", bufs=2))
        opool = ctx.enter_context(tc.tile_pool(name="acts", bufs=1))
        outpool = ctx.enter_context(tc.tile_pool(name="outs", bufs=1))
        ppool = ctx.enter_context(tc.tile_pool(name="psum", bufs=4, space="PSUM"))

        XDT = E4 if fp8 else BF16
        for ch in range(NCHUNK * rep):
            r0 = (ch % NCHUNK) * R
            xt_f = xpool.tile([BS, NB, R], F32, tag="xt_f", name=f"xt_f{ch}")
            for ci in range(NB):
                nc.sync.dma_start(out=xt_f[:, ci, :], in_=xt[ci, :, r0:r0 + R])
            xt_q = bfpool.tile([BS, NB, R], XDT, tag="xt_q", name=f"xt_q{ch}")
            if fp8:
                nc.gpsimd.tensor_scalar_mul(xt_q[:], xt_f[:], SX)
            else:
                nc.gpsimd.tensor_copy(xt_q[:], xt_f[:])

            # layer 1 (FFT fused): o1s = relu(psum + b1s); scale folded into b1s/w2
            o1r = opool.tile([BS, NB, R], BF16, tag="o1r", name=f"o1r{ch}")
            o1i = opool.tile([BS, NB, R], BF16, tag="o1i", name=f"o1i{ch}")
            for ro in range(NB):
                pr = ppool.tile([BS, R], F32, tag="pr", name=f"pr{ch}_{ro}")
                pi = ppool.tile([BS, R], F32, tag="pi", name=f"pi{ch}_{ro}")
                cs = slice(ro * BS, (ro + 1) * BS)
                if fp8:
                    for t in range(NB // 2):
                        nc.tensor.matmul(pr[:], w1r_sb[:, 2 * t:2 * t + 2, cs],
                                         xt_q[:, 2 * t:2 * t + 2, :],
                                         start=(t == 0), stop=(t == NB // 2 - 1),
                                         perf_mode=DR)
                    for t in range(NB // 2):
                        nc.tensor.matmul(pi[:], w1i_sb[:, 2 * t:2 * t + 2, cs],
                                         xt_q[:, 2 * t:2 * t + 2, :],
                                         start=(t == 0), stop=(t == NB // 2 - 1),
                                         perf_mode=DR)
                else:
                    for ci in range(NB):
                        nc.tensor.matmul(pr[:], w1r_sb[:, ci, cs], xt_q[:, ci, :],
                                         start=(ci == 0), stop=(ci == NB - 1))
                    for ci in range(NB):
                        nc.tensor.matmul(pi[:], w1i_sb[:, ci, cs], xt_q[:, ci, :],
                                         start=(ci == 0), stop=(ci == NB - 1))
                nc.scalar.activation(o1r[:, ro, :], pr[:], RELU,
                                     bias=bias_tiles["b1r"][:, ro:ro + 1])
                nc.vector.tensor_scalar(o1i[:, ro, :], pi[:],
                                        bias_tiles["b1i"][:, ro:ro + 1], 0.0,
                                        ALU.add, ALU.max)

            # layer 2 (block-diag, bf16) + softshrink via z - clamp(z, +-lam)
            o2r = opool.tile([BS, NB, R], XDT, tag="o2r", name=f"o2r{ch}")
            o2i = opool.tile([BS, NB, R], XDT, tag="o2i", name=f"o2i{ch}")
            for kb in range(NB):
                qr = ppool.tile([BS, R], F32, tag="pr", name=f"qr{ch}_{kb}")
                qi = ppool.tile([BS, R], F32, tag="pi", name=f"qi{ch}_{kb}")
                nc.tensor.matmul(qr[:], w20_sb[:, kb, :], o1r[:, kb, :],
                                 start=True, stop=False)
                nc.tensor.matmul(qr[:], w21n_sb[:, kb, :], o1i[:, kb, :],
                                 start=False, stop=True)
                nc.tensor.matmul(qi[:], w20_sb[:, kb, :], o1i[:, kb, :],
                                 start=True, stop=False)
                nc.tensor.matmul(qi[:], w21_sb[:, kb, :], o1r[:, kb, :],
                                 start=False, stop=True)
                ur = bfpool.tile([BS, R], BF16, tag="ur", name=f"ur{ch}_{kb}")
                ui = bfpool.tile([BS, R], BF16, tag="ui", name=f"ui{ch}_{kb}")
                vr = bfpool.tile([BS, R], BF16, tag="vr", name=f"vr{ch}_{kb}")
                vi = bfpool.tile([BS, R], BF16, tag="vi", name=f"vi{ch}_{kb}")
                nc.scalar.activation(ur[:], qr[:],
                                     mybir.ActivationFunctionType.Identity,
                                     bias=bias_tiles["b2r"][:, kb:kb + 1])
                nc.vector.tensor_scalar_add(ui[:], qi[:], bias_tiles["b2i"][:, kb:kb + 1])
                nc.gpsimd.tensor_scalar(vr[:], ur[:], -LAMS, LAMS, ALU.max, ALU.min)
                nc.gpsimd.tensor_scalar(vi[:], ui[:], -LAMS, LAMS, ALU.max, ALU.min)
                nc.vector.tensor_sub(o2r[:, kb, :], ur[:], vr[:])
                nc.gpsimd.tensor_sub(o2i[:, kb, :], ui[:], vi[:])

            # layer 3 (IFFT real part): p3 = SO*SG*(o2r@cos + o2i@(-sin));
            # out = K3*p3 + x
            out_f = outpool.tile([BS, NB, R], F32, tag="out_f", name=f"out_f{ch}")
            for co in range(NB):
                p3 = ppool.tile([BS, R], F32, tag="pr", name=f"p3_{ch}_{co}")
                cs = slice(co * BS, (co + 1) * BS)
                if fp8:
                    for t in range(NB // 2):
                        nc.tensor.matmul(p3[:], gc_sb[:, 2 * t:2 * t + 2, cs],
                                         o2r[:, 2 * t:2 * t + 2, :],
                                         start=(t == 0), stop=False, perf_mode=DR)
                    for t in range(NB // 2):
                        nc.tensor.matmul(p3[:], gs_sb[:, 2 * t:2 * t + 2, cs],
                                         o2i[:, 2 * t:2 * t + 2, :],
                                         start=False, stop=(t == NB // 2 - 1),
                                         perf_mode=DR)
                else:
                    for kb in range(NB):
                        nc.tensor.matmul(p3[:], gc_sb[:, kb, cs], o2r[:, kb, :],
                                         start=(kb == 0), stop=False)
                    for kb in range(NB):
                        nc.tensor.matmul(p3[:], gs_sb[:, kb, cs], o2i[:, kb, :],
                                         start=False, stop=(kb == NB - 1))
                nc.vector.scalar_tensor_tensor(out_f[:, co, :], p3[:], K3, xt_f[:, co, :],
                                               ALU.mult, ALU.add)
                nc.sync.dma_start(out=out[co, :, r0:r0 + R], in_=out_f[:, co, :])

    nc.compile()
    _GRAPH_CACHE[key] = nc
    return nc


def _build_host_weights(w1, b1, w2, b2, fp8=FP8):
    C = HIDDEN
    k = np.arange(C)
    c = np.arange(C)
    ph = (np.outer(c, k) % C).astype(np.float64) * (2.0 * np.pi / C)
    s = 1.0 / np.sqrt(C)
    Fr = np.cos(ph) * s        # [c, k]
    Fi = -np.sin(ph) * s
    w1 = np.asarray(w1, np.float64)
    W1r = np.empty((C, C), np.float64)
    W1i = np.empty((C, C), np.float64)
    for kb in range(NB):
        cols = slice(kb * BS, (kb + 1) * BS)
        W1r[:, cols] = Fr[:, cols] @ w1[0, kb] - Fi[:, cols] @ w1[1, kb]
        W1i[:, cols] = Fi[:, cols] @ w1[0, kb] + Fr[:, cols] @ w1[1, kb]
    # IFFT (real part): out = o2r @ Gr + o2i @ Gi, G[k, c]
    Gr = Fr.T.copy()           # cos(2pi k c / C)/sqrt(C)
    Gi = Fi.T.copy()           # -sin(2pi k c / C)/sqrt(C)

    b1 = np.asarray(b1, np.float64)
    b2 = np.asarray(b2, np.float64)
    w2 = np.asarray(w2, np.float64)
    bf = ml_dtypes.bfloat16
    f8 = ml_dtypes.float8_e4m3

    if fp8:
        wmax = max(np.abs(W1r).max(), np.abs(W1i).max())
        sw1 = 2.0 ** np.floor(np.log2(120.0 / wmax))
        s1 = SX * sw1                          # scale of o1 activations
        sw2 = SO / s1                          # folds o1/o2 scales into w2
        out = {
            "w1r": np.ascontiguousarray((W1r * sw1).reshape(NB, BS, HIDDEN)).astype(f8),
            "w1i": np.ascontiguousarray((W1i * sw1).reshape(NB, BS, HIDDEN)).astype(f8),
            "gc": np.ascontiguousarray((Gr * SG).reshape(NB, BS, HIDDEN)).astype(f8),
            "gs": np.ascontiguousarray((Gi * SG).reshape(NB, BS, HIDDEN)).astype(f8),
            "w20": (w2[0] * sw2).astype(np.float32).astype(bf),
            "w21": (w2[1] * sw2).astype(np.float32).astype(bf),
            "w21n": (-w2[1] * sw2).astype(np.float32).astype(bf),
            "b1r": np.ascontiguousarray((b1[0] * s1).T).astype(np.float32),
            "b1i": np.ascontiguousarray((b1[1] * s1).T).astype(np.float32),
            "b2r": np.ascontiguousarray((b2[0] * SO).T).astype(np.float32),
            "b2i": np.ascontiguousarray((b2[1] * SO).T).astype(np.float32),
        }
    else:
        out = {
            "w1r": np.ascontiguousarray(W1r.reshape(NB, BS, HIDDEN)).astype(bf),
            "w1i": np.ascontiguousarray(W1i.reshape(NB, BS, HIDDEN)).astype(bf),
            "gc": np.ascontiguousarray(Gr.reshape(NB, BS, HIDDEN)).astype(bf),
            "gs": np.ascontiguousarray(Gi.reshape(NB, BS, HIDDEN)).astype(bf),
            "w20": w2[0].astype(np.float32).astype(bf),
            "w21": w2[1].astype(np.float32).astype(bf),
            "w21n": (-w2[1]).astype(np.float32).astype(bf),
            "b1r": np.ascontiguousarray(b1[0].T).astype(np.float32),
            "b1i": np.ascontiguousarray(b1[1].T).astype(np.float32),
            "b2r": np.ascontiguousarray(b2[0].T).astype(np.float32),
            "b2i": np.ascontiguousarray(b2[1].T).astype(np.float32),
        }
    return out


def _make_in_maps(x, w1, b1, w2, b2):
    x = np.asarray(x, np.float32)
    B = x.shape[0]
    weights = _build_host_weights(w1, b1, w2, b2)
    in_maps = []
    for b in range(B):
        m = dict(weights)
        m["xt"] = np.ascontiguousarray(x[b].T).reshape(NB, BS, NROWS)
        in_maps.append(m)
    return in_maps


def _run(x, w1, b1, w2, b2, trace=False):
    nc = _build_graph()
    x = np.asarray(x, np.float32)
    B = x.shape[0]
    in_maps = _make_in_maps(x, w1, b1, w2, b2)
    res = run_bass_kernel_spmd(nc, in_maps, core_ids=list(range(N_CORES)),
                               trace=trace)
    outs = np.empty_like(x)
    for b in range(B):
        outs[b] = res.results[b]["out"].reshape(HIDDEN, NROWS).T
    return outs, res


def kernel(x, w1, b1, w2, b2):
    outs, _ = _run(x, w1, b1, w2, b2, trace=False)
    return outs


# revision 12
# speedup vs baseline: 4.4044x; 4.4044x over previous
"""AFNO1D Trainium2 kernel: FFT->block-MLP->softshrink->IFFT->residual.

Strategy: the FFT along C is linear, so it is fused into the layer-1
weights on the host (W1_eff = DFT_block @ w1); the IFFT's real output is
built from half-spectrum matmuls A = o2r@cos, B = o2i@sin with
out[c] = A-B+x and out[1024-c] = A+B+x[1024-c] (reversal done by host
indexing). Everything on-chip is matmul + elementwise, computed
channel-major so the contraction dim sits on SBUF partitions.

Data-parallel over B=8: core b handles x[b]; params replicated; no
collectives. Host transposes shards in/out.

Compute dtype: fp8(e4m3) operands in DoubleRow mode for the two dense
layers, bf16 for the small block-diagonal layer, fp32 PSUM + fp32
residual path. All quantization scales fold into host-prepared weights
and biases; the graph itself has only two fixed descale constants.
"""

from contextlib import ExitStack

import numpy as np
import ml_dtypes

import concourse.bass as bass
import concourse.mybir as mybir
import concourse.tile as tile
from concourse import bacc
from concourse.bass_utils import run_bass_kernel_spmd

HIDDEN = 1024
NB = 8          # channel blocks
BS = 128        # block size
LAM = 0.01
N_CORES = 8
NROWS = 4096    # rows (sequence positions) per core
R = 512         # rows per chunk
NCHUNK = NROWS // R

FP8 = True      # fp8 DoubleRow for layers 1/3 (bf16 fallback if False)
SX = 8.0        # x -> fp8 scale (|x| up to 30 before clipping)
SO = 32.0       # o2 -> fp8 scale (|o2| up to 7.5)
SG = 2048.0     # IFFT cos/sin -> fp8 scale (entries <= 1/32 -> <= 64)
ALU = mybir.AluOpType

F32 = mybir.dt.float32
BF16 = mybir.dt.bfloat16
E4 = mybir.dt.float8e4
DR = mybir.MatmulPerfMode.DoubleRow
RELU = mybir.ActivationFunctionType.Relu

_GRAPH_CACHE = {}


def _build_graph(rep=1, fp8=FP8):
    key = ("nc", rep, fp8)
    if key in _GRAPH_CACHE:
        return _GRAPH_CACHE[key]

    WDT = E4 if fp8 else BF16

    nc = bacc.Bacc("TRN2", target_bir_lowering=False, debug=False,
                   num_devices=N_CORES)

    xt = nc.dram_tensor("xt", [NB, BS, NROWS], F32, kind="ExternalInput").ap()
    xt8 = nc.dram_tensor("xt8", [NB, BS, NROWS], E4 if fp8 else BF16,
                         kind="ExternalInput").ap()
    w1r = nc.dram_tensor("w1r", [NB, BS, HIDDEN], WDT, kind="ExternalInput").ap()
    w1i = nc.dram_tensor("w1i", [NB, BS, HIDDEN], WDT, kind="ExternalInput").ap()
    gc = nc.dram_tensor("gc", [NB, BS, HIDDEN], WDT, kind="ExternalInput").ap()
    gs = nc.dram_tensor("gs", [NB, BS, HIDDEN], WDT, kind="ExternalInput").ap()
    w20 = nc.dram_tensor("w20", [NB, BS, BS], BF16, kind="ExternalInput").ap()
    w21 = nc.dram_tensor("w21", [NB, BS, BS], BF16, kind="ExternalInput").ap()
    w21n = nc.dram_tensor("w21n", [NB, BS, BS], BF16, kind="ExternalInput").ap()
    b1r = nc.dram_tensor("b1r", [BS, NB], F32, kind="ExternalInput").ap()
    b1i = nc.dram_tensor("b1i", [BS, NB], F32, kind="ExternalInput").ap()
    b2r = nc.dram_tensor("b2r", [BS, NB], F32, kind="ExternalInput").ap()
    b2i = nc.dram_tensor("b2i", [BS, NB], F32, kind="ExternalInput").ap()
    out = nc.dram_tensor("out", [NB, BS, NROWS], F32, kind="ExternalOutput").ap()

    # graph descale constants (everything else is folded host-side)
    K3 = 1.0 / (SO * SG) if fp8 else 1.0
    K5 = 1.0 / (SO * 32.0) if fp8 else 1.0
    LAMS = (SO if fp8 else 1.0) * LAM

    with tile.TileContext(nc) as tc, ExitStack() as ctx:
        wpool = ctx.enter_context(tc.tile_pool(name="weights", bufs=1))
        w1r_sb = wpool.tile([BS, NB, HIDDEN], WDT, tag="w1r", name="w1r_sb")
        w1i_sb = wpool.tile([BS, NB, HIDDEN], WDT, tag="w1i", name="w1i_sb")
        gc_sb = wpool.tile([BS, NB, HIDDEN], WDT, tag="gc", name="gc_sb")
        gs_sb = wpool.tile([BS, NB, HIDDEN], WDT, tag="gs", name="gs_sb")
        for ci in range(NB):
            nc.sync.dma_start(out=w1r_sb[:, ci, :], in_=w1r[ci])
            nc.sync.dma_start(out=w1i_sb[:, ci, :], in_=w1i[ci])
            nc.sync.dma_start(out=gc_sb[:, ci, :], in_=gc[ci])
            nc.sync.dma_start(out=gs_sb[:, ci, :], in_=gs[ci])
        w20_sb = wpool.tile([BS, NB, BS], BF16, tag="w20", name="w20_sb")
        w21_sb = wpool.tile([BS, NB, BS], BF16, tag="w21", name="w21_sb")
        w21n_sb = wpool.tile([BS, NB, BS], BF16, tag="w21n", name="w21n_sb")
        for kb in range(NB):
            nc.sync.dma_start(out=w20_sb[:, kb, :], in_=w20[kb])
            nc.sync.dma_start(out=w21_sb[:, kb, :], in_=w21[kb])
            nc.sync.dma_start(out=w21n_sb[:, kb, :], in_=w21n[kb])
        bias_tiles = {}
        for nm, ap in (("b1r", b1r), ("b1i", b1i), ("b2r", b2r), ("b2i", b2i)):
            t = wpool.tile([BS, NB], F32, tag=nm, name=f"{nm}_sb")
            nc.sync.dma_start(out=t[:], in_=ap[:])
            bias_tiles[nm] = t

        xpool = ctx.enter_context(tc.tile_pool(name="xin", bufs=2))
        bfpool = ctx.enter_context(tc.tile_pool(name="bf", bufs=2))
        opool = ctx.enter_context(tc.tile_pool(name="acts", bufs=1))
        outpool = ctx.enter_context(tc.tile_pool(name="outs", bufs=1))
        ppool = ctx.enter_context(tc.tile_pool(name="psum", bufs=4, space="PSUM"))

        XDT = E4 if fp8 else BF16
        for ch in range(NCHUNK * rep):
            r0 = (ch % NCHUNK) * R
            xt_f = xpool.tile([BS, NB, R], F32, tag="xt_f", name=f"xt_f{ch}")
            for ci in range(NB):
                nc.sync.dma_start(out=xt_f[:, ci, :], in_=xt[ci, :, r0:r0 + R])
            xt_q = bfpool.tile([BS, NB, R], XDT, tag="xt_q", name=f"xt_q{ch}")
            for ci in range(NB):
                nc.sync.dma_start(out=xt_q[:, ci, :], in_=xt8[ci, :, r0:r0 + R])

            # layer 1 (FFT fused): o1s = relu(psum + b1s); scale folded into b1s/w2
            o1r = opool.tile([BS, NB, R], BF16, tag="o1r", name=f"o1r{ch}")
            o1i = opool.tile([BS, NB, R], BF16, tag="o1i", name=f"o1i{ch}")
            for ro in range(NB):
                pr = ppool.tile([BS, R], F32, tag="pr", name=f"pr{ch}_{ro}")
                pi = ppool.tile([BS, R], F32, tag="pi", name=f"pi{ch}_{ro}")
                cs = slice(ro * BS, (ro + 1) * BS)
                if fp8:
                    for t in range(NB // 2):
                        nc.tensor.matmul(pr[:], w1r_sb[:, 2 * t:2 * t + 2, cs],
                                         xt_q[:, 2 * t:2 * t + 2, :],
                                         start=(t == 0), stop=(t == NB // 2 - 1),
                                         perf_mode=DR)
                    for t in range(NB // 2):
                        nc.tensor.matmul(pi[:], w1i_sb[:, 2 * t:2 * t + 2, cs],
                                         xt_q[:, 2 * t:2 * t + 2, :],
                                         start=(t == 0), stop=(t == NB // 2 - 1),
                                         perf_mode=DR)
                else:
                    for ci in range(NB):
                        nc.tensor.matmul(pr[:], w1r_sb[:, ci, cs], xt_q[:, ci, :],
                                         start=(ci == 0), stop=(ci == NB - 1))
                    for ci in range(NB):
                        nc.tensor.matmul(pi[:], w1i_sb[:, ci, cs], xt_q[:, ci, :],
                                         start=(ci == 0), stop=(ci == NB - 1))
                nc.scalar.activation(o1r[:, ro, :], pr[:], RELU,
                                     bias=bias_tiles["b1r"][:, ro:ro + 1])
                nc.vector.tensor_scalar(o1i[:, ro, :], pi[:],
                                        bias_tiles["b1i"][:, ro:ro + 1], 0.0,
                                        ALU.add, ALU.max)

            # layer 2 (block-diag, bf16) + softshrink via z - clamp(z, +-lam)
            o2r = opool.tile([BS, NB, R], XDT, tag="o2r", name=f"o2r{ch}")
            o2i = opool.tile([BS, NB, R], XDT, tag="o2i", name=f"o2i{ch}")
            for kb in range(NB):
                qr = ppool.tile([BS, R], F32, tag="pr", name=f"qr{ch}_{kb}")
                qi = ppool.tile([BS, R], F32, tag="pi", name=f"qi{ch}_{kb}")
                nc.tensor.matmul(qr[:], w20_sb[:, kb, :], o1r[:, kb, :],
                                 start=True, stop=False)
                nc.tensor.matmul(qr[:], w21n_sb[:, kb, :], o1i[:, kb, :],
                                 start=False, stop=True)
                nc.tensor.matmul(qi[:], w20_sb[:, kb, :], o1i[:, kb, :],
                                 start=True, stop=False)
                nc.tensor.matmul(qi[:], w21_sb[:, kb, :], o1r[:, kb, :],
                                 start=False, stop=True)
                ur = bfpool.tile([BS, R], BF16, tag="ur", name=f"ur{ch}_{kb}")
                ui = bfpool.tile([BS, R], BF16, tag="ui", name=f"ui{ch}_{kb}")
                vr = bfpool.tile([BS, R], BF16, tag="vr", name=f"vr{ch}_{kb}")
                vi = bfpool.tile([BS, R], BF16, tag="vi", name=f"vi{ch}_{kb}")
                nc.scalar.activation(ur[:], qr[:],
                                     mybir.ActivationFunctionType.Identity,
                                     bias=bias_tiles["b2r"][:, kb:kb + 1])
                nc.scalar.activation(ui[:], qi[:],
                                     mybir.ActivationFunctionType.Identity,
                                     bias=bias_tiles["b2i"][:, kb:kb + 1])
                nc.vector.tensor_scalar(vr[:], ur[:], -LAMS, LAMS, ALU.max, ALU.min)
                nc.vector.tensor_scalar(vi[:], ui[:], -LAMS, LAMS, ALU.max, ALU.min)
                nc.vector.tensor_sub(o2r[:, kb, :], ur[:], vr[:])
                nc.vector.tensor_sub(o2i[:, kb, :], ui[:], vi[:])

            # layer 3 (IFFT real part): p3 = SO*SG*(o2r@cos + o2i@(-sin));
            # out = K3*p3 + x
            out_f = outpool.tile([BS, NB, R], F32, tag="out_f", name=f"out_f{ch}")
            for co in range(NB):
                p3 = ppool.tile([BS, R], F32, tag="pr", name=f"p3_{ch}_{co}")
                cs = slice(co * BS, (co + 1) * BS)
                if fp8:
                    for t in range(NB // 2):
                        nc.tensor.matmul(p3[:], gc_sb[:, 2 * t:2 * t + 2, cs],
                                         o2r[:, 2 * t:2 * t + 2, :],
                                         start=(t == 0), stop=False, perf_mode=DR)
                    for t in range(NB // 2):
                        nc.tensor.matmul(p3[:], gs_sb[:, 2 * t:2 * t + 2, cs],
                                         o2i[:, 2 * t:2 * t + 2, :],
                                         start=False, stop=(t == NB // 2 - 1),
                                         perf_mode=DR)
                else:
                    for kb in range(NB):
                        nc.tensor.matmul(p3[:], gc_sb[:, kb, cs], o2r[:, kb, :],
                                         start=(kb == 0), stop=False)
                    for kb in range(NB):
                        nc.tensor.matmul(p3[:], gs_sb[:, kb, cs], o2i[:, kb, :],
                                         start=False, stop=(kb == NB - 1))
                nc.vector.scalar_tensor_tensor(out_f[:, co, :], p3[:], K3, xt_f[:, co, :],
                                               ALU.mult, ALU.add)
                nc.sync.dma_start(out=out[co, :, r0:r0 + R], in_=out_f[:, co, :])

    nc.compile()
    _GRAPH_CACHE[key] = nc
    return nc


def _build_host_weights(w1, b1, w2, b2, fp8=FP8):
    C = HIDDEN
    k = np.arange(C)
    c = np.arange(C)
    ph = (np.outer(c, k) % C).astype(np.float64) * (2.0 * np.pi / C)
    s = 1.0 / np.sqrt(C)
    Fr = np.cos(ph) * s        # [c, k]
    Fi = -np.sin(ph) * s
    w1 = np.asarray(w1, np.float64)
    W1r = np.empty((C, C), np.float64)
    W1i = np.empty((C, C), np.float64)
    for kb in range(NB):
        cols = slice(kb * BS, (kb + 1) * BS)
        W1r[:, cols] = Fr[:, cols] @ w1[0, kb] - Fi[:, cols] @ w1[1, kb]
        W1i[:, cols] = Fi[:, cols] @ w1[0, kb] + Fr[:, cols] @ w1[1, kb]
    # IFFT (real part): out = o2r @ Gr + o2i @ Gi, G[k, c]
    Gr = Fr.T.copy()           # cos(2pi k c / C)/sqrt(C)
    Gi = Fi.T.copy()           # -sin(2pi k c / C)/sqrt(C)

    b1 = np.asarray(b1, np.float64)
    b2 = np.asarray(b2, np.float64)
    w2 = np.asarray(w2, np.float64)
    bf = ml_dtypes.bfloat16
    f8 = ml_dtypes.float8_e4m3

    if fp8:
        wmax = max(np.abs(W1r).max(), np.abs(W1i).max())
        sw1 = 2.0 ** np.floor(np.log2(120.0 / wmax))
        s1 = SX * sw1                          # scale of o1 activations
        sw2 = SO / s1                          # folds o1/o2 scales into w2
        out = {
            "w1r": np.ascontiguousarray((W1r * sw1).reshape(NB, BS, HIDDEN)).astype(f8),
            "w1i": np.ascontiguousarray((W1i * sw1).reshape(NB, BS, HIDDEN)).astype(f8),
            "gc": np.ascontiguousarray((Gr * SG).reshape(NB, BS, HIDDEN)).astype(f8),
            "gs": np.ascontiguousarray((Gi * SG).reshape(NB, BS, HIDDEN)).astype(f8),
            "w20": (w2[0] * sw2).astype(np.float32).astype(bf),
            "w21": (w2[1] * sw2).astype(np.float32).astype(bf),
            "w21n": (-w2[1] * sw2).astype(np.float32).astype(bf),
            "b1r": np.ascontiguousarray((b1[0] * s1).T).astype(np.float32),
            "b1i": np.ascontiguousarray((b1[1] * s1).T).astype(np.float32),
            "b2r": np.ascontiguousarray((b2[0] * SO).T).astype(np.float32),
            "b2i": np.ascontiguousarray((b2[1] * SO).T).astype(np.float32),
        }
    else:
        out = {
            "w1r": np.ascontiguousarray(W1r.reshape(NB, BS, HIDDEN)).astype(bf),
            "w1i": np.ascontiguousarray(W1i.reshape(NB, BS, HIDDEN)).astype(bf),
            "gc": np.ascontiguousarray(Gr.reshape(NB, BS, HIDDEN)).astype(bf),
            "gs": np.ascontiguousarray(Gi.reshape(NB, BS, HIDDEN)).astype(bf),
            "w20": w2[0].astype(np.float32).astype(bf),
            "w21": w2[1].astype(np.float32).astype(bf),
            "w21n": (-w2[1]).astype(np.float32).astype(bf),
            "b1r": np.ascontiguousarray(b1[0].T).astype(np.float32),
            "b1i": np.ascontiguousarray(b1[1].T).astype(np.float32),
            "b2r": np.ascontiguousarray(b2[0].T).astype(np.float32),
            "b2i": np.ascontiguousarray(b2[1].T).astype(np.float32),
        }
    return out


def _make_in_maps(x, w1, b1, w2, b2):
    x = np.asarray(x, np.float32)
    B = x.shape[0]
    weights = _build_host_weights(w1, b1, w2, b2)
    in_maps = []
    qdt = ml_dtypes.float8_e4m3 if FP8 else ml_dtypes.bfloat16
    qs = SX if FP8 else 1.0
    for b in range(B):
        m = dict(weights)
        xt_b = np.ascontiguousarray(x[b].T)
        m["xt"] = xt_b.reshape(NB, BS, NROWS)
        m["xt8"] = (xt_b * qs).astype(qdt).reshape(NB, BS, NROWS)
        in_maps.append(m)
    return in_maps


def _run(x, w1, b1, w2, b2, trace=False):
    nc = _build_graph()
    x = np.asarray(x, np.float32)
    B = x.shape[0]
    in_maps = _make_in_maps(x, w1, b1, w2, b2)
    res = run_bass_kernel_spmd(nc, in_maps, core_ids=list(range(N_CORES)),
                               trace=trace)
    outs = np.empty_like(x)
    for b in range(B):
        outs[b] = res.results[b]["out"].reshape(HIDDEN, NROWS).T
    return outs, res


def kernel(x, w1, b1, w2, b2):
    outs, _ = _run(x, w1, b1, w2, b2, trace=False)
    return outs


# revision 14
# speedup vs baseline: 5.8202x; 1.3215x over previous
"""AFNO1D Trainium2 kernel: FFT->block-MLP->softshrink->IFFT->residual.

Strategy: the FFT along C is linear, so it is fused into the layer-1
weights on the host (W1_eff = DFT_block @ w1); the IFFT's real output is
built from half-spectrum matmuls A = o2r@cos, B = o2i@sin with
out[c] = A-B+x and out[1024-c] = A+B+x[1024-c] (reversal done by host
indexing). Everything on-chip is matmul + elementwise, computed
channel-major so the contraction dim sits on SBUF partitions.

Data-parallel over B=8: core b handles x[b]; params replicated; no
collectives. Host transposes shards in/out.

Compute dtype: fp8(e4m3) operands in DoubleRow mode for the two dense
layers, bf16 for the small block-diagonal layer, fp32 PSUM + fp32
residual path. All quantization scales fold into host-prepared weights
and biases; the graph itself has only two fixed descale constants.
"""

from contextlib import ExitStack

import numpy as np
import ml_dtypes

import concourse.bass as bass
import concourse.mybir as mybir
import concourse.tile as tile
from concourse import bacc
from concourse.bass_utils import run_bass_kernel_spmd

HIDDEN = 1024
NB = 8          # channel blocks
BS = 128        # block size
LAM = 0.01
N_CORES = 8
NROWS = 4096    # rows (sequence positions) per core
R = 512         # rows per chunk
NCHUNK = NROWS // R

FP8 = True      # fp8 DoubleRow for layers 1/3 (bf16 fallback if False)
SX = 8.0        # x -> fp8 scale (|x| up to 30 before clipping)
SW1 = 512.0     # W1_eff -> fp8 scale (|W1| up to 0.47)
SO = 32.0       # o1 and o2 -> fp8 scale factor (via SW2 = SO)
SG = 2048.0     # IFFT cos/sin -> fp8 scale (entries <= 1/32 -> <= 64)
ALU = mybir.AluOpType

F32 = mybir.dt.float32
BF16 = mybir.dt.bfloat16
E4 = mybir.dt.float8e4
DR = mybir.MatmulPerfMode.DoubleRow
RELU = mybir.ActivationFunctionType.Relu

_GRAPH_CACHE = {}


def _build_graph(rep=1, fp8=FP8):
    key = ("nc", rep, fp8)
    if key in _GRAPH_CACHE:
        return _GRAPH_CACHE[key]

    WDT = E4 if fp8 else BF16

    nc = bacc.Bacc("TRN2", target_bir_lowering=False, debug=False,
                   num_devices=N_CORES)

    xt = nc.dram_tensor("xt", [NB, BS, NROWS], F32, kind="ExternalInput").ap()
    xt8 = nc.dram_tensor("xt8", [NB, BS, NROWS], E4 if fp8 else BF16,
                         kind="ExternalInput").ap()
    w1r = nc.dram_tensor("w1r", [NB, BS, HIDDEN], WDT, kind="ExternalInput").ap()
    w1i = nc.dram_tensor("w1i", [NB, BS, HIDDEN], WDT, kind="ExternalInput").ap()
    gp = nc.dram_tensor("gp", [NB, BS, 2, HIDDEN], WDT, kind="ExternalInput").ap()
    w2rp = nc.dram_tensor("w2rp", [NB, BS, 2, BS], WDT, kind="ExternalInput").ap()
    w2ip = nc.dram_tensor("w2ip", [NB, BS, 2, BS], WDT, kind="ExternalInput").ap()
    b1r = nc.dram_tensor("b1r", [BS, NB], F32, kind="ExternalInput").ap()
    b1i = nc.dram_tensor("b1i", [BS, NB], F32, kind="ExternalInput").ap()
    b2m1r = nc.dram_tensor("b2m1r", [BS, NB], F32, kind="ExternalInput").ap()
    b2m1i = nc.dram_tensor("b2m1i", [BS, NB], F32, kind="ExternalInput").ap()
    b2m2r = nc.dram_tensor("b2m2r", [BS, NB], F32, kind="ExternalInput").ap()
    b2m2i = nc.dram_tensor("b2m2i", [BS, NB], F32, kind="ExternalInput").ap()
    out = nc.dram_tensor("out", [NB, BS, NROWS], F32, kind="ExternalOutput").ap()

    # graph descale constants (everything else is folded host-side)
    K3 = 1.0 / (SO * SG) if fp8 else 1.0
    SIG = 1.0 / (SX * SW1) if fp8 else 1.0   # L1 psum descale (o1 at true scale)

    with tile.TileContext(nc) as tc, ExitStack() as ctx:
        wpool = ctx.enter_context(tc.tile_pool(name="weights", bufs=1))
        w1r_sb = wpool.tile([BS, NB, HIDDEN], WDT, tag="w1r", name="w1r_sb")
        w1i_sb = wpool.tile([BS, NB, HIDDEN], WDT, tag="w1i", name="w1i_sb")
        gp_sb = wpool.tile([BS, NB, 2, HIDDEN], WDT, tag="gp", name="gp_sb")
        for ci in range(NB):
            nc.sync.dma_start(out=w1r_sb[:, ci, :], in_=w1r[ci])
            nc.sync.dma_start(out=w1i_sb[:, ci, :], in_=w1i[ci])
            nc.sync.dma_start(out=gp_sb[:, ci, :, :], in_=gp[ci])
        w2rp_sb = wpool.tile([BS, NB, 2, BS], WDT, tag="w2rp", name="w2rp_sb")
        w2ip_sb = wpool.tile([BS, NB, 2, BS], WDT, tag="w2ip", name="w2ip_sb")
        for kb in range(NB):
            nc.sync.dma_start(out=w2rp_sb[:, kb, :, :], in_=w2rp[kb])
            nc.sync.dma_start(out=w2ip_sb[:, kb, :, :], in_=w2ip[kb])
        bias_tiles = {}
        for nm, ap in (("b1r", b1r), ("b1i", b1i), ("b2m1r", b2m1r),
                       ("b2m1i", b2m1i), ("b2m2r", b2m2r), ("b2m2i", b2m2i)):
            t = wpool.tile([BS, NB], F32, tag=nm, name=f"{nm}_sb")
            nc.sync.dma_start(out=t[:], in_=ap[:])
            bias_tiles[nm] = t

        xpool = ctx.enter_context(tc.tile_pool(name="xin", bufs=2))
        bfpool = ctx.enter_context(tc.tile_pool(name="bf", bufs=2))
        opool = ctx.enter_context(tc.tile_pool(name="acts", bufs=1))
        outpool = ctx.enter_context(tc.tile_pool(name="outs", bufs=1))
        ppool = ctx.enter_context(tc.tile_pool(name="psum", bufs=4, space="PSUM"))

        XDT = E4 if fp8 else BF16
        for ch in range(NCHUNK * rep):
            r0 = (ch % NCHUNK) * R
            xt_f = xpool.tile([BS, NB, R], F32, tag="xt_f", name=f"xt_f{ch}")
            for ci in range(NB):
                nc.sync.dma_start(out=xt_f[:, ci, :], in_=xt[ci, :, r0:r0 + R])
            xt_q = bfpool.tile([BS, NB, R], XDT, tag="xt_q", name=f"xt_q{ch}")
            for ci in range(NB):
                nc.sync.dma_start(out=xt_q[:, ci, :], in_=xt8[ci, :, r0:r0 + R])

            # layer 1 (FFT fused): o1 = relu(SIG*psum + b1), stored paired fp8
            o1p = opool.tile([BS, NB, 2, R], XDT, tag="o1p", name=f"o1p{ch}")
            for ro in range(NB):
                pr = ppool.tile([BS, R], F32, tag="pr", name=f"pr{ch}_{ro}")
                pi = ppool.tile([BS, R], F32, tag="pi", name=f"pi{ch}_{ro}")
                cs = slice(ro * BS, (ro + 1) * BS)
                if fp8:
                    for t in range(NB // 2):
                        nc.tensor.matmul(pr[:], w1r_sb[:, 2 * t:2 * t + 2, cs],
                                         xt_q[:, 2 * t:2 * t + 2, :],
                                         start=(t == 0), stop=(t == NB // 2 - 1),
                                         perf_mode=DR)
                    for t in range(NB // 2):
                        nc.tensor.matmul(pi[:], w1i_sb[:, 2 * t:2 * t + 2, cs],
                                         xt_q[:, 2 * t:2 * t + 2, :],
                                         start=(t == 0), stop=(t == NB // 2 - 1),
                                         perf_mode=DR)
                else:
                    for ci in range(NB):
                        nc.tensor.matmul(pr[:], w1r_sb[:, ci, cs], xt_q[:, ci, :],
                                         start=(ci == 0), stop=(ci == NB - 1))
                    for ci in range(NB):
                        nc.tensor.matmul(pi[:], w1i_sb[:, ci, cs], xt_q[:, ci, :],
                                         start=(ci == 0), stop=(ci == NB - 1))
                nc.scalar.activation(o1p[:, ro, 0, :], pr[:], RELU, scale=SIG,
                                     bias=bias_tiles["b1r"][:, ro:ro + 1])
                nc.scalar.activation(o1p[:, ro, 1, :], pi[:], RELU, scale=SIG,
                                     bias=bias_tiles["b1i"][:, ro:ro + 1])

            # layer 2 (block-diag complex, fp8 DR on paired o1) + softshrink:
            # softshrink(z) = relu(z + b2 - lam) - relu(-z - b2 - lam), z = SO*o2pre
            o2p = opool.tile([BS, NB, 2, R], XDT, tag="o2p", name=f"o2p{ch}")
            for kb in range(NB):
                qr = ppool.tile([BS, R], F32, tag="pr", name=f"qr{ch}_{kb}")
                qi = ppool.tile([BS, R], F32, tag="pi", name=f"qi{ch}_{kb}")
                if fp8:
                    nc.tensor.matmul(qr[:], w2rp_sb[:, kb, :, :], o1p[:, kb, :, :],
                                     start=True, stop=True, perf_mode=DR)
                    nc.tensor.matmul(qi[:], w2ip_sb[:, kb, :, :], o1p[:, kb, :, :],
                                     start=True, stop=True, perf_mode=DR)
                else:
                    nc.tensor.matmul(qr[:], w2rp_sb[:, kb, 0, :], o1p[:, kb, 0, :],
                                     start=True, stop=False)
                    nc.tensor.matmul(qr[:], w2rp_sb[:, kb, 1, :], o1p[:, kb, 1, :],
                                     start=False, stop=True)
                    nc.tensor.matmul(qi[:], w2ip_sb[:, kb, 0, :], o1p[:, kb, 0, :],
                                     start=True, stop=False)
                    nc.tensor.matmul(qi[:], w2ip_sb[:, kb, 1, :], o1p[:, kb, 1, :],
                                     start=False, stop=True)
                t1r = bfpool.tile([BS, R], BF16, tag="t1r", name=f"t1r{ch}_{kb}")
                t1i = bfpool.tile([BS, R], BF16, tag="t1i", name=f"t1i{ch}_{kb}")
                t2r = bfpool.tile([BS, R], BF16, tag="t2r", name=f"t2r{ch}_{kb}")
                t2i = bfpool.tile([BS, R], BF16, tag="t2i", name=f"t2i{ch}_{kb}")
                nc.vector.tensor_scalar(t1r[:], qr[:],
                                        bias_tiles["b2m1r"][:, kb:kb + 1], 0.0,
                                        ALU.add, ALU.max)
                nc.vector.tensor_scalar(t1i[:], qi[:],
                                        bias_tiles["b2m1i"][:, kb:kb + 1], 0.0,
                                        ALU.add, ALU.max)
                nc.scalar.activation(t2r[:], qr[:], RELU, scale=-1.0,
                                     bias=bias_tiles["b2m2r"][:, kb:kb + 1])
                nc.scalar.activation(t2i[:], qi[:], RELU, scale=-1.0,
                                     bias=bias_tiles["b2m2i"][:, kb:kb + 1])
                nc.vector.tensor_sub(o2p[:, kb, 0, :], t1r[:], t2r[:])
                nc.vector.tensor_sub(o2p[:, kb, 1, :], t1i[:], t2i[:])

            # layer 3 (IFFT real part): p3 = SO*SG*(o2r@cos + o2i@(-sin))
            # via paired (Gr|Gi) against (o2r|o2i); out = K3*p3 + x
            out_f = outpool.tile([BS, NB, R], F32, tag="out_f", name=f"out_f{ch}")
            for co in range(NB):
                p3 = ppool.tile([BS, R], F32, tag="pr", name=f"p3_{ch}_{co}")
                cs = slice(co * BS, (co + 1) * BS)
                if fp8:
                    for kb in range(NB):
                        nc.tensor.matmul(p3[:], gp_sb[:, kb, :, cs],
                                         o2p[:, kb, :, :],
                                         start=(kb == 0), stop=(kb == NB - 1),
                                         perf_mode=DR)
                else:
                    for kb in range(NB):
                        nc.tensor.matmul(p3[:], gp_sb[:, kb, 0, cs], o2p[:, kb, 0, :],
                                         start=(kb == 0), stop=False)
                    for kb in range(NB):
                        nc.tensor.matmul(p3[:], gp_sb[:, kb, 1, cs], o2p[:, kb, 1, :],
                                         start=False, stop=(kb == NB - 1))
                nc.vector.scalar_tensor_tensor(out_f[:, co, :], p3[:], K3, xt_f[:, co, :],
                                               ALU.mult, ALU.add)
                nc.sync.dma_start(out=out[co, :, r0:r0 + R], in_=out_f[:, co, :])

    nc.compile()
    _GRAPH_CACHE[key] = nc
    return nc


def _build_host_weights(w1, b1, w2, b2, fp8=FP8):
    C = HIDDEN
    k = np.arange(C)
    c = np.arange(C)
    ph = (np.outer(c, k) % C).astype(np.float64) * (2.0 * np.pi / C)
    s = 1.0 / np.sqrt(C)
    Fr = np.cos(ph) * s        # [c, k]
    Fi = -np.sin(ph) * s
    w1 = np.asarray(w1, np.float64)
    W1r = np.empty((C, C), np.float64)
    W1i = np.empty((C, C), np.float64)
    for kb in range(NB):
        cols = slice(kb * BS, (kb + 1) * BS)
        W1r[:, cols] = Fr[:, cols] @ w1[0, kb] - Fi[:, cols] @ w1[1, kb]
        W1i[:, cols] = Fi[:, cols] @ w1[0, kb] + Fr[:, cols] @ w1[1, kb]
    # IFFT (real part): out = o2r @ Gr + o2i @ Gi, G[k, c]
    Gr = Fr.T.copy()           # cos(2pi k c / C)/sqrt(C)
    Gi = Fi.T.copy()           # -sin(2pi k c / C)/sqrt(C)

    b1 = np.asarray(b1, np.float64)
    b2 = np.asarray(b2, np.float64)
    w2 = np.asarray(w2, np.float64)
    bf = ml_dtypes.bfloat16
    f8 = ml_dtypes.float8_e4m3

    if fp8:
        s1 = SX * SW1
        f8w = f8
    else:
        s1 = 1.0
        SW1_, SO_, SG_ = 1.0, 1.0, 1.0
        f8w = bf
    sw1 = SW1 if fp8 else 1.0
    so = SO if fp8 else 1.0
    sg = SG if fp8 else 1.0

    def pack_pairs(a, b):
        # [NB, BS, X] x2 -> [NB, BS, 2, X]
        return np.ascontiguousarray(np.stack([a, b], axis=2))

    Grb = (Gr * sg).reshape(NB, BS, HIDDEN)
    Gib = (Gi * sg).reshape(NB, BS, HIDDEN)
    out = {
        "w1r": np.ascontiguousarray((W1r * sw1).reshape(NB, BS, HIDDEN)).astype(f8w),
        "w1i": np.ascontiguousarray((W1i * sw1).reshape(NB, BS, HIDDEN)).astype(f8w),
        "gp": pack_pairs(Grb, Gib).astype(f8w),
        "w2rp": pack_pairs(w2[0] * so, -w2[1] * so).astype(f8w),
        "w2ip": pack_pairs(w2[1] * so, w2[0] * so).astype(f8w),
        "b1r": np.ascontiguousarray(b1[0].T).astype(np.float32),
        "b1i": np.ascontiguousarray(b1[1].T).astype(np.float32),
        "b2m1r": np.ascontiguousarray((so * (b2[0] - LAM)).T).astype(np.float32),
        "b2m1i": np.ascontiguousarray((so * (b2[1] - LAM)).T).astype(np.float32),
        "b2m2r": np.ascontiguousarray((so * (-b2[0] - LAM)).T).astype(np.float32),
        "b2m2i": np.ascontiguousarray((so * (-b2[1] - LAM)).T).astype(np.float32),
    }
    return out


def _make_in_maps(x, w1, b1, w2, b2):
    x = np.asarray(x, np.float32)
    B = x.shape[0]
    weights = _build_host_weights(w1, b1, w2, b2)
    in_maps = []
    qdt = ml_dtypes.float8_e4m3 if FP8 else ml_dtypes.bfloat16
    qs = SX if FP8 else 1.0
    for b in range(B):
        m = dict(weights)
        xt_b = np.ascontiguousarray(x[b].T)
        m["xt"] = xt_b.reshape(NB, BS, NROWS)
        m["xt8"] = (xt_b * qs).astype(qdt).reshape(NB, BS, NROWS)
        in_maps.append(m)
    return in_maps


def _run(x, w1, b1, w2, b2, trace=False):
    nc = _build_graph()
    x = np.asarray(x, np.float32)
    B = x.shape[0]
    in_maps = _make_in_maps(x, w1, b1, w2, b2)
    res = run_bass_kernel_spmd(nc, in_maps, core_ids=list(range(N_CORES)),
                               trace=trace)
    outs = np.empty_like(x)
    for b in range(B):
        outs[b] = res.results[b]["out"].reshape(HIDDEN, NROWS).T
    return outs, res


def kernel(x, w1, b1, w2, b2):
    outs, _ = _run(x, w1, b1, w2, b2, trace=False)
    return outs
